# revision 5
# baseline (speedup 1.0000x reference)
"""Trainium2 Bass kernel for DequantingLinear (GGML Q8_0 dequant + linear).

Computes out[4096, 12288] = x[4096, 3072] @ dequant(w_q, w_scales).T + bias
where w_q is int32 (int8-valued) with per-32-element-block fp32 scales.

Sharding: tensor-parallel over output features across 8 NeuronCores. Each
core gets the full x and a 1536-row shard of w_q / w_scales / bias,
computes its [4096, 1536] output slice; the host concatenates on axis 1.

Per-core kernel (Tile framework), v4 — no DRAM bounces:
  * w path: w_q int32 row-chunks load via SWDGE (gpsimd ring, parallel to
    the sync ring), DVE dequant (int32 x block-broadcast fp32 scales ->
    bf16, exact for |q|<=127), then ONE SBUF->SBUF xbar transpose per
    half-chunk into the resident wt[in-part, k, out] tensor.
  * x path: natural fp32 half-row loads on the sync ring, ACT cast to
    bf16, SBUF->SBUF xbar transpose into per-(block,m) xt tiles
    [128, 24, 128]. No DRAM round-trip, no SWDGE cast.
  * GEMM: psum[128 tok, 512 out] tiles accumulate 24 bf16 k-tile matmuls
    (fp32 PSUM); bias is added during the PSUM->SBUF copy on the vector
    engine. Phase-1 (n=0 for the first two token blocks) starts as soon
    as w chunks 0-3 and the first xt tiles land.
  * x loads for blocks >= 2 are paced behind the last w xbar so bulk x
    traffic cannot clog the sync HWDGE ring ahead of the w chain.
  HBM traffic/core: x 50.3 MB + w_q 18.9 MB + out 25.2 MB (+0.6); the
  34.6 MB of transposes ride the SBUF fabric, not HBM.
"""

import sys

for _p in ("/opt/trn_rl_repo",):
    if _p not in sys.path:
        sys.path.append(_p)

from contextlib import ExitStack

import numpy as np

import concourse.bacc as bacc
import concourse.bass as bass
import concourse.mybir as mybir
from concourse import tile
from concourse.tile_rust import add_dep_helper
from concourse.bass_utils import run_bass_kernel_spmd

FP32 = mybir.dt.float32
BF16 = mybir.dt.bfloat16
INT32 = mybir.dt.int32

N_CORES = 8
TOK, IN, OUT = 4096, 3072, 12288
QK = 32
OUT_SH = OUT // N_CORES
TOK_BLK = 512
NCOL = 512
NB1 = 2
HALF = IN // 2


def _build(nc: bass.Bass, repeats: int = 1, serialize: bool = False):
    P = 128
    KT = IN // P
    NBLK = TOK // TOK_BLK
    MT = TOK_BLK // P
    NT = OUT_SH // NCOL
    NB = IN // QK
    OT = OUT_SH // P
    KH = KT // 2

    x = nc.dram_tensor("x", [TOK, IN], FP32, kind="ExternalInput")
    w_q = nc.dram_tensor("w_q", [OUT_SH, IN], INT32, kind="ExternalInput")
    w_scales = nc.dram_tensor("w_scales", [OUT_SH, NB], FP32, kind="ExternalInput")
    bias = nc.dram_tensor("bias", [OUT_SH], FP32, kind="ExternalInput")
    out = nc.dram_tensor("out", [TOK, OUT_SH], FP32, kind="ExternalOutput")

    prev_last = None
    with tile.TileContext(nc) as tc:
      for _rep in range(repeats):
       with ExitStack() as ctx:
        const_pool = ctx.enter_context(tc.tile_pool(name=f"const{_rep}", bufs=1))
        wq_pool = ctx.enter_context(tc.tile_pool(name=f"wq{_rep}", bufs=3))
        wd_pool = ctx.enter_context(tc.tile_pool(name=f"wd{_rep}", bufs=2))
        wt_pool = ctx.enter_context(tc.tile_pool(name=f"wt{_rep}", bufs=1))
        xf_pool = ctx.enter_context(tc.tile_pool(name=f"xf{_rep}", bufs=3))
        xb_pool = ctx.enter_context(tc.tile_pool(name=f"xb{_rep}", bufs=3))
        xt_pool = ctx.enter_context(tc.tile_pool(name=f"xt{_rep}", bufs=9))
        out_pool = ctx.enter_context(tc.tile_pool(name=f"out{_rep}", bufs=4))
        psum_pool = ctx.enter_context(
            tc.tile_pool(name=f"psum{_rep}", bufs=8, space="PSUM")
        )

        entries = []

        sc_tiles = []
        for o in range(OT):
            sct = const_pool.tile([P, NB], FP32, tag=f"sc_{o}")
            entries.append(nc.sync.dma_start(sct[:], w_scales.ap()[o * P : (o + 1) * P, :]))
            sc_tiles.append(sct)

        bias_rep = const_pool.tile([P, OUT_SH], FP32, tag="bias_rep")
        entries.append(
            nc.sync.dma_start(
                bias_rep[:], bias.ap().unsqueeze(0).to_broadcast([P, OUT_SH])
            )
        )

        wt = wt_pool.tile([P, KT, OUT_SH], BF16, tag="wt")
        last_w_xbar = None

        def w_chunk(o):
            nonlocal last_w_xbar
            rows = slice(o * P, (o + 1) * P)
            wd = wd_pool.tile([P, IN], BF16, tag="wd")
            for h in range(2):
                cols = slice(h * HALF, (h + 1) * HALF)
                wq_i = wq_pool.tile([P, HALF], INT32, tag="wq")
                entries.append(nc.gpsimd.dma_start(wq_i[:], w_q.ap()[rows, cols]))
                nc.vector.tensor_mul(
                    wd[:, cols].rearrange("p (b q) -> p b q", q=QK),
                    wq_i[:].rearrange("p (b q) -> p b q", q=QK),
                    sc_tiles[o][:, h * (NB // 2) : (h + 1) * (NB // 2)]
                    .unsqueeze(2)
                    .to_broadcast([P, NB // 2, QK]),
                )
                last_w_xbar = nc.sync.dma_start(
                    wt[:, h * KH : (h + 1) * KH, o * P : (o + 1) * P],
                    wd[:, cols],
                    transpose=True,
                )

        def load_xt(b, m, paced):
            tok0 = b * TOK_BLK + m * P
            xt_m = xt_pool.tile([P, KT, P], BF16, tag="xt")
            for h in range(2):
                cols = slice(h * HALF, (h + 1) * HALF)
                xf = xf_pool.tile([P, HALF], FP32, tag="xf")
                ld = nc.sync.dma_start(xf[:], x.ap()[tok0 : tok0 + P, cols])
                if paced:
                    add_dep_helper(
                        ld.ins, last_w_xbar.ins, reason="pace x loads behind w-prep"
                    )
                else:
                    entries.append(ld)
                xb = xb_pool.tile([P, HALF], BF16, tag="xb")
                nc.scalar.copy(xb[:], xf[:])
                nc.sync.dma_start(
                    xt_m[:, h * KH : (h + 1) * KH, :], xb[:], transpose=True
                )
            return xt_m

        # Head: w chunks for the n=0 column block, then phase-1 xt tiles.
        for o in range(NCOL // P):
            w_chunk(o)

        xt_tiles = {}
        for b in range(NB1):
            for m in range(MT):
                xt_tiles[(b, m)] = load_xt(b, m, paced=False)

        def gemm_group(xt_m, b, m, n):
            tok0 = b * TOK_BLK + m * P
            ps = psum_pool.tile([P, NCOL], FP32, tag="ps")
            for k in range(KT):
                nc.tensor.matmul(
                    ps[:],
                    xt_m[:, k, :],
                    wt[:, k, n * NCOL : (n + 1) * NCOL],
                    start=(k == 0),
                    stop=(k == KT - 1),
                )
            ob = out_pool.tile([P, NCOL], FP32, tag="ob")
            nc.vector.tensor_add(ob[:], ps[:], bias_rep[:, n * NCOL : (n + 1) * NCOL])
            return nc.sync.dma_start(
                out.ap()[tok0 : tok0 + P, n * NCOL : (n + 1) * NCOL], ob[:]
            )

        # Phase-1 GEMM: n=0 for the first NB1 blocks.
        for b in range(NB1):
            for m in range(MT):
                gemm_group(xt_tiles[(b, m)], b, m, 0)

        # Remaining w chunks.
        for o in range(NCOL // P, OT):
            w_chunk(o)

        # Main loop with one-block x prefetch.
        last_store = None
        for b in range(NBLK):
            nb = b + 1
            if NB1 <= nb < NBLK:
                for m in range(MT):
                    xt_tiles[(nb, m)] = load_xt(nb, m, paced=True)
            for m in range(MT):
                xt_m = xt_tiles.pop((b, m))
                for n in range(NT):
                    if b < NB1 and n == 0:
                        continue
                    last_store = gemm_group(xt_m, b, m, n)

        if serialize and prev_last is not None:
            for e in entries:
                add_dep_helper(e.ins, prev_last.ins, reason="serialize reps")
        prev_last = last_store
    return nc


_COMPILED_NC = None


def _get_nc():
    global _COMPILED_NC
    if _COMPILED_NC is None:
        nc = bacc.Bacc("TRN2", target_bir_lowering=False, debug=False)
        _build(nc)
        nc.compile()
        _COMPILED_NC = nc
    return _COMPILED_NC


def kernel(x, w_q, w_scales, bias):
    assert x.shape == (TOK, IN) and w_q.shape == (OUT, IN)
    nc = _get_nc()
    x = np.ascontiguousarray(np.asarray(x, dtype=np.float32))
    w_q = np.asarray(w_q, dtype=np.int32)
    w_scales = np.asarray(w_scales, dtype=np.float32)
    bias = np.asarray(bias, dtype=np.float32)
    in_maps = []
    for c in range(N_CORES):
        r = slice(c * OUT_SH, (c + 1) * OUT_SH)
        in_maps.append(
            {
                "x": x,
                "w_q": np.ascontiguousarray(w_q[r]),
                "w_scales": np.ascontiguousarray(w_scales[r]),
                "bias": np.ascontiguousarray(bias[r]),
            }
        )
    res = run_bass_kernel_spmd(nc, in_maps, list(range(N_CORES)))
    return np.concatenate([res.results[c]["out"] for c in range(N_CORES)], axis=1)
